# revision 1
# baseline (speedup 1.0000x reference)
"""Trainium2 Bass kernel for nn_EquiCtsConvBase (equivariant continuous conv).

Math reformulation (per batch b, center m, field point n):
  rel = (field[n] - center[m]) / RADIUS
  r, theta = polar(rel)
  Bilinear grid-sample of kernel[(co,ci,y,x), theta_pad, r] at
  (gx, gy) decomposes into separable hat functions:
    Wx[j]  = relu(1 - |4r - 0.5 - j|)            j = 0..3   (radius cells)
    Wy[l]  = relu(1 - |iy - l|), iy = 4*theta/pi + 4.5, l = 0..9
  Circular theta padding folds 10 rows -> 8 bins:
    Wy8[0] = max(Wy[1], Wy[9]); Wy8[7] = max(Wy[0], Wy[8]); Wy8[b]=Wy[b+1]
  att = relu(1 - |rel|^2)^3 * mask[n]   (mask folded into feat on host)
  A[(b8,j), n, m] = relu(Wx_pre[j]) * relu(Wy8_pre[b8]) * att
  G[m, cell, f]   = sum_n A[cell, n, m] * feat[n, f]        (PE matmul 1)
  out[m, (co,y)]  = sum_{cell,f} G * K2[cell, f, (co,y)]    (PE matmul 2)
  out /= max(psi, tiny), psi[m] = sum_n att[n, m]  (extra ones-column matmul)

theta is computed without a Sqrt (keeps a single ACT table, trig_and_small):
  phi = arctan(rely/relx);  theta = phi + pi*sign(rely)*[relx<0]
  r   = |relx*sin(phi+pi/2) + rely*sin(phi)|

Sharding: 8 cores; core c handles batch b = c//4, centers m0 = (c%4)*96 .. +96.
Each core's SPMD program is identical; only input data differs.
"""

import math
import numpy as np

RADIUS = 1.5
B, M, N = 2, 384, 384
CI = CO = 8
M_LOC = 96          # centers per core
NCH = 3             # n-chunks of 128 (N = 384)
NCELL = 32          # 8 theta bins x 4 radius cells
FREE = NCH * M_LOC  # 288: fused (chunk, m) free dim for elementwise ops
N_CORES = 8

# --- engine assignment tuning knobs ---
CFG = dict(
    wy_act=tuple(range(0, 8)),   # Wy hat indices computed on ACT (rest on DVE)
    n_a_gps=6,                   # how many of the 32 A-cell ops go to GPSIMD
    wya_gps=True,                # wya (8 stt ops) on GPSIMD
    use_bcast=True,              # stride-0 free-dim broadcast reads
    a_dtype="f32",               # dtype for A / feat matmul operands
    streams=1,
)

_module_cache = {}


def _build_module(cfg):
    import concourse.bass as bass
    import concourse.bacc as bacc
    import concourse.mybir as mybir
    from concourse import tile

    dt = mybir.dt
    Alu = mybir.AluOpType
    Act = mybir.ActivationFunctionType

    nc = bacc.Bacc("TRN2", target_bir_lowering=False, debug=False,
                   num_devices=N_CORES)

    # Register activation-bias constants as const APs (memset + barrier,
    # same as Bass's built-ins) so ACT ops don't need a DMA sync wait.
    _eng_rr = [nc.gpsimd, nc.vector]

    def _register_const(value):
        key = (dt.float32, float(value))
        if key in nc.const_aps.aps:
            return
        t = nc.alloc_sbuf_tensor(
            f"kcst-{len(nc.const_aps.aps)}", [128, 1], dt.float32)
        _eng_rr[len(nc.const_aps.aps) % 2].memset(t.ap(), float(value))
        nc.const_aps.aps[key] = t.ap()

    for _v in ([-float(l) for l in range(1, 8)]
               + [-(0.5 + j) for j in range(4)] + [math.pi / 2]):
        _register_const(_v)
    nc.all_engine_barrier()

    # ------------- DRAM I/O -------------
    # single fused input for all elementwise consumers (one DMA -> one wait):
    # cols 0..191 = cx|cy (/RADIUS, per m), 192..197 = fx|fy per chunk,
    # 198..213 = bias constants
    inpd = nc.dram_tensor("inp", [128, 214], dt.float32,
                          kind="ExternalInput").ap()
    feat_dt = (dt.bfloat16 if CFG["a_dtype"] == "bf16" else dt.float32r)
    featd = nc.dram_tensor("featx", [128, NCH * 33], feat_dt,
                           kind="ExternalInput").ap()
    k2d = nc.dram_tensor("k2b", [16, NCELL * 16], dt.float32,
                         kind="ExternalInput").ap()
    outd = nc.dram_tensor("out", [M_LOC, 16], dt.float32,
                          kind="ExternalOutput").ap()

    f32 = dt.float32
    f32r = dt.float32r

    with tile.TileContext(nc) as tc:
        with tc.tile_pool(name="p", bufs=1) as pool, \
             tc.tile_pool(name="ps", bufs=1, space="PSUM") as psum:

            # ---------- loads ----------
            inp_s = pool.tile([128, 214], f32, tag="inp", name="inp_s")
            feat_s = pool.tile([128, NCH * 33], feat_dt, tag="feat", name="feat_s")
            k2_s = pool.tile([16, NCELL * 16], f32, tag="k2", name="k2_s")
            nc.sync.dma_start(inp_s[:], inpd[:])
            nc.sync.dma_start(feat_s[:], featd[:])
            nc.sync.dma_start(k2_s[:], k2d[:])

            # const bias columns: 0..9 -> -l (Wy), 10..13 -> -(0.5+j) (Wx),
            # 14 -> pi/2, 15 -> 1.0
            def cB(i):
                return inp_s[:, 198 + i:199 + i]

            def wt(tag, shape=None):
                return pool.tile(shape or [128, NCH, M_LOC], f32, tag=tag,
                                 name=tag)

            # broadcast views [128, NCH, M_LOC]
            cb_sl_x = inp_s[:, None, 0:M_LOC].to_broadcast((128, NCH, M_LOC))
            cb_sl_y = inp_s[:, None, M_LOC:2 * M_LOC].to_broadcast(
                (128, NCH, M_LOC))
            fx_b = inp_s[:, 192:195, None].to_broadcast((128, NCH, M_LOC))
            fy_b = inp_s[:, 195:198, None].to_broadcast((128, NCH, M_LOC))

            V, S, G = nc.vector, nc.scalar, nc.gpsimd

            # warm-up: pin the trig_and_small ACT table (the only table with
            # Sin+Arctan+Abs+Square) so exactly one table load happens, early
            warm = pool.tile([1, 1], f32, tag="warm", name="warm")
            zc = nc.const_aps.aps[(dt.float32, 0.0)][0:1]
            S.activation(warm[:], zc, Act.Sin)
            S.activation(warm[:], zc, Act.Arctan)

            # ---------- elementwise stage (split into independent m-
            # streams so DVE/ACT/Pool chains pipeline) ----------
            relx = wt("relx"); rely = wt("rely")
            sqx = wt("sqx"); sqy = wt("sqy"); rho = wt("rho")
            rx = wt("rx"); ry = wt("ry"); t1 = wt("t1"); t2 = wt("t2")
            swp = pool.tile([128, NCH, M_LOC], dt.int32, tag="swp", name="swp")
            sab = pool.tile([128, NCH, M_LOC], dt.int32, tag="sab", name="sab")
            phi = wt("phi"); psw = wt("psw")
            sgn = wt("sgn"); neg = wt("neg"); sgx = wt("sgx"); sgt = wt("sgt")
            cs = wt("cs"); sn = wt("sn")
            xc = wt("xc"); ys = wt("ys"); rr = wt("rr")
            corr = wt("corr"); phis = wt("phis"); iy = wt("iy")
            u1 = wt("u1"); u2 = wt("u2")
            adt = dt.bfloat16 if cfg["a_dtype"] == "bf16" else f32r
            a_t = pool.tile([128, NCELL + 1, NCH, M_LOC], adt, tag="a_t",
                            name="a_t")
            att = wt("att")
            wyh = pool.tile([128, 10, NCH, M_LOC], f32, tag="wyh", name="wyh")
            w0s = wt("w0s"); w7s = wt("w7s")
            wya = pool.tile([128, 8, NCH, M_LOC], adt, tag="wya", name="wya")
            wxp = pool.tile([128, 4, NCH, M_LOC], f32, tag="wxp", name="wxp")
            wxr = pool.tile([128, 4, NCH, M_LOC], adt, tag="wxr", name="wxr")
            hat_a1 = wt("hat_a1")

            n_streams = cfg.get("streams", 1)
            step = M_LOC // n_streams
            for st in range(n_streams):
                lo, hi = st * step, (st + 1) * step

                def sl(t):
                    return t[:, :, lo:hi]

                def sl4(t, i):
                    return t[:, i, :, lo:hi]

                cxs = cb_sl_x[:, :, lo:hi]
                cys = cb_sl_y[:, :, lo:hi]
                fxs = fx_b[:, :, lo:hi]
                fys = fy_b[:, :, lo:hi]

                V.tensor_tensor(sl(relx), fxs, cxs, Alu.subtract)
                V.tensor_tensor(sl(rely), fys, cys, Alu.subtract)
                S.activation(sl(sqx), sl(relx), Act.Square)
                S.activation(sl(sqy), sl(rely), Act.Square)
                V.tensor_tensor(sl(rho), sl(sqx), sl(sqy), Alu.add)

                V.reciprocal(sl(rx), sl(relx))
                V.reciprocal(sl(ry), sl(rely))
                V.tensor_tensor(sl(t1), sl(rely), sl(rx), Alu.mult)
                V.tensor_tensor(sl(t2), sl(relx), sl(ry), Alu.mult)
                V.tensor_scalar(sl(sab), sl(t1).bitcast(dt.int32), 0x7FFFFFFF,
                                None, Alu.bitwise_and)
                V.tensor_scalar(sl(swp), sl(sab).bitcast(f32), 1.0, None,
                                Alu.is_gt)
                V.tensor_copy(sl(phi), sl(t1))
                V.copy_predicated(sl(phi), sl(swp), sl(t2))
                S.activation(sl(phi), sl(phi), Act.Arctan)
                S.activation(sl(sgn), sl(rely), Act.Sign)
                V.tensor_scalar(sl(neg), sl(relx), 0.0, None, Alu.is_lt)
                S.activation(sl(sgt), sl(t1), Act.Sign)  # sign(y/x)
                V.scalar_tensor_tensor(sl(psw), sl(sgt), math.pi / 2, sl(phi),
                                       Alu.mult, Alu.subtract)
                V.copy_predicated(sl(phi), sl(swp), sl(psw))

                S.activation(sl(cs), sl(phi), Act.Sin, bias=math.pi / 2)
                S.activation(sl(sn), sl(phi), Act.Sin)
                V.tensor_tensor(sl(xc), sl(relx), sl(cs), Alu.mult)
                V.tensor_tensor(sl(ys), sl(rely), sl(sn), Alu.mult)
                V.tensor_tensor(sl(rr), sl(xc), sl(ys), Alu.add)
                V.tensor_scalar(sl(rr).bitcast(dt.int32),
                                sl(rr).bitcast(dt.int32),
                                0x7FFFFFFF, None, Alu.bitwise_and)  # |.|

                V.tensor_tensor(sl(corr), sl(sgn), sl(neg), Alu.mult)
                V.tensor_scalar(sl(phis), sl(phi), 4.0 / math.pi, 4.5,
                                Alu.mult, Alu.add)
                V.scalar_tensor_tensor(sl(iy), sl(corr), 4.0, sl(phis),
                                       Alu.mult, Alu.add)

                S.activation(sl(u1), sl(rho), Act.Relu, bias=1.0, scale=-1.0)
                S.activation(sl(u2), sl(u1), Act.Square)
                V.scalar_tensor_tensor(sl(att), sl(u2), 1.0, sl(u1),
                                       Alu.mult, Alu.mult)  # u1^3
                V.tensor_copy(a_t[:, NCELL, :, lo:hi], sl(att))

                # Wy hats (pre-relu): 1 - |iy - l|
                for l in range(10):
                    dst = sl4(wyh, l)
                    if l in cfg["wy_act"]:
                        S.activation(dst, sl(iy), Act.Abs, bias=float(-l))
                        S.activation(dst, dst, Act.Identity, bias=1.0,
                                     scale=-1.0)
                    else:
                        V.tensor_scalar(sl(hat_a1), sl(iy), -1.0,
                                        float(1 + l), Alu.mult, Alu.add)
                        V.tensor_scalar(dst, sl(iy), 1.0, float(1 - l),
                                        Alu.mult, Alu.add)
                        V.tensor_tensor(dst, dst, sl(hat_a1), Alu.min)

                V.tensor_tensor(sl(w0s), sl4(wyh, 1), sl4(wyh, 9), Alu.max)
                V.tensor_tensor(sl(w7s), sl4(wyh, 0), sl4(wyh, 8), Alu.max)

                def wy8_pre(b8):
                    if b8 == 0:
                        return sl(w0s)
                    if b8 == 7:
                        return sl(w7s)
                    return sl4(wyh, b8 + 1)

                for b8 in range(8):
                    V.scalar_tensor_tensor(sl4(wya, b8), wy8_pre(b8),
                                           0.0, sl(att), Alu.max, Alu.mult)

                # Wx pre-relu hats on ACT, then relu'd copy
                for j in range(4):
                    dst = sl4(wxp, j)
                    S.activation(dst, sl(rr), Act.Abs, bias=-(0.5 + j),
                                 scale=4.0)
                    S.activation(dst, dst, Act.Identity, bias=1.0, scale=-1.0)
                    V.tensor_scalar(sl4(wxr, j), sl4(wxp, j), 0.0, None,
                                    Alu.max)

                # early cells on DVE (fast) so matmul-1 groups unblock
                # sooner; late cells on the slower Pool engine
                n_gps = cfg["n_a_gps"]
                for cell in range(NCELL):
                    b8, j = divmod(cell, 4)
                    eng = G if cell >= NCELL - n_gps else V
                    eng.tensor_tensor(a_t[:, cell, :, lo:hi], sl4(wxr, j),
                                      sl4(wya, b8), Alu.mult)

            # ---------- matmul 1: G = featx^T @ A  (accumulate over chunks)
            groups = [(0, 5), (5, 10), (10, 15), (15, 20), (20, 25),
                      (25, 30), (30, 33)]
            g_ps = []
            for gi, (c0, c1) in enumerate(groups):
                g_ps.append(psum.tile([33, (c1 - c0) * M_LOC], f32,
                                      tag=f"g{gi}", name=f"g{gi}"))
            for u in range(NCH):
                lhs = feat_s[:, u * 33:(u + 1) * 33]
                for gi, (c0, c1) in enumerate(groups):
                    rhs = a_t[:, c0:c1, u, :]
                    nc.tensor.matmul(g_ps[gi][:], lhs, rhs,
                                     start=(u == 0), stop=(u == NCH - 1))

            # ---------- psi -> 1/psi, transposed to [96, 1] ----------
            psi_ap = g_ps[6][32:33, 2 * M_LOC:3 * M_LOC]  # [1, 96]
            psir = pool.tile([1, M_LOC], f32, tag="psir", name="psir")
            V.tensor_scalar(psir[:], psi_ap, 1e-35, None, Alu.max)
            V.reciprocal(psir[:], psir[:])
            psit = pool.tile([M_LOC, 1], f32, tag="psit", name="psit")
            nc.sync.dma_start(psit[:, 0:1], psir[0:1, :])

            # ---------- G PSUM -> SBUF (fat aligned copies) ----------
            gs = pool.tile([16, NCELL * M_LOC], f32, tag="gs", name="gs")
            for gi, (c0, c1) in enumerate(groups):
                w = (min(c1, NCELL) - c0) * M_LOC
                dst = gs[:, c0 * M_LOC:c0 * M_LOC + w]
                if gi in (0, 2, 4):
                    V.tensor_copy(dst, g_ps[gi][0:16, 0:w])
                else:
                    S.activation(dst, g_ps[gi][0:16, 0:w], Act.Copy)

            # ---------- matmul 2 (transposed): o2t[m, coy] ----------
            o2t = psum.tile([M_LOC, 16], f32, tag="o2t", name="o2t")
            for c in range(NCELL):
                nc.tensor.matmul(o2t[:],
                                 gs[:, c * M_LOC:(c + 1) * M_LOC],
                                 k2_s[:, c * 16:(c + 1) * 16],
                                 start=(c == 0), stop=(c == NCELL - 1))

            # ---------- scale by 1/psi, store ----------
            out_s = pool.tile([M_LOC, 16], f32, tag="outs", name="out_s")
            V.tensor_scalar(out_s[:], o2t[:], psit[:, 0:1], None, Alu.mult)
            nc.sync.dma_start(outd[:], out_s[:])

    nc.compile()
    return nc


def get_module(cfg=None):
    cfg = dict(CFG, **(cfg or {}))
    key = tuple(sorted((k, str(v)) for k, v in cfg.items()))
    if key not in _module_cache:
        _module_cache[key] = _build_module(cfg)
    return _module_cache[key]


def make_in_maps(field, center, field_feat, field_mask, kernel, cfg=None):
    """Host-side shard + layout prep. Returns list of 8 in_maps."""
    cfg = dict(CFG, **(cfg or {}))
    field = np.asarray(field, np.float32)
    center = np.asarray(center, np.float32)
    feat = np.asarray(field_feat, np.float32)
    mask = np.asarray(field_mask, np.float32)
    ker = np.asarray(kernel, np.float32)

    # K2big: [128 rows = (cell%8)*16 + (ci*2+x), 64 cols = (cell//8)*16 + (co*2+y)]
    kk = ker.transpose(3, 2, 1, 5, 0, 4).reshape(NCELL, 16, 16)  # [cell,(ci,x),(co,y)]
    k2b = np.ascontiguousarray(kk.transpose(1, 0, 2).reshape(16, NCELL * 16),
                               np.float32)

    in_maps = []
    for c in range(N_CORES):
        b, blk = divmod(c, 4)
        m0 = blk * M_LOC
        cx = center[b, m0:m0 + M_LOC, 0] / RADIUS   # [96]
        cy = center[b, m0:m0 + M_LOC, 1] / RADIUS
        fx = (field[b, :, 0] / RADIUS).reshape(NCH, 128).T  # [128, 3]
        fy = (field[b, :, 1] / RADIUS).reshape(NCH, 128).T

        cst_row = np.array([-l for l in range(10)]
                           + [-(0.5 + j) for j in range(4)]
                           + [math.pi / 2, 1.0], np.float32)
        inp = np.concatenate([np.broadcast_to(cx, (128, M_LOC)),
                              np.broadcast_to(cy, (128, M_LOC)),
                              fx, fy,
                              np.broadcast_to(cst_row, (128, 16))], axis=1)

        fm = feat[b].reshape(N, 16) * mask[b]           # mask folded
        # [N, 33]: 16 feat, 16 zero pad, ones*mask col (psi lands at row 32)
        fcols = np.concatenate([fm, np.zeros((N, 16), np.float32), mask[b]],
                               axis=1)
        featx = fcols.reshape(NCH, 128, 33).transpose(1, 0, 2).reshape(128, 99)

        if cfg["a_dtype"] == "bf16":
            import ml_dtypes
            featx_c = np.ascontiguousarray(featx).astype(ml_dtypes.bfloat16)
        else:
            featx_c = np.ascontiguousarray(featx, np.float32)
        in_maps.append({
            "inp": np.ascontiguousarray(inp, np.float32),
            "featx": featx_c,
            "k2b": k2b,
        })
    return in_maps


def unshard(results):
    out = np.zeros((B, M, CO, 2), np.float32)
    for c in range(N_CORES):
        b, blk = divmod(c, 4)
        m0 = blk * M_LOC
        out[b, m0:m0 + M_LOC] = results[c]["out"].reshape(M_LOC, CO, 2)
    return out


def kernel(field, center, field_feat, field_mask, kernel):
    from concourse.bass_utils import run_bass_kernel_spmd
    nc = get_module()
    in_maps = make_in_maps(field, center, field_feat, field_mask, kernel)
    res = run_bass_kernel_spmd(nc, in_maps, core_ids=list(range(N_CORES)))
    return unshard(res.results)

